# revision 1
# baseline (speedup 1.0000x reference)
"""DeepseekV3Attention on 8 Trainium2 NeuronCores (Bass/Tile).

Sharding: DP2 over batch x TP4 over heads. Core c handles batch c//4 and
heads 4*(c%4)..4*(c%4)+3. Each core computes a partial o_proj output
[S, HID]; the host sums the 4 partials per batch.

Per-core pipeline (all matmuls fp32r = fp32 data at full PE rate for
moving-dim >= 256):

Stage 1 (per s-tile of 512):
  A) a-proj:  q_aT (12 M-blocks), ckvT (4), k-rope il/sw duplicated
     blocks (2); contraction over 16 hid-chunks; psum groups of 6.
     ACT squares + all-ones stationary matmuls give column sums of
     squares broadcast over partitions -> rmsnorm scales rs_q/rs_kv
     as [128, ST] tiles (all partitions identical).
  B) q b-proj: qT blocks = WqbT'.T @ q_a_rawT; rs_q applied during the
     psum->sbuf drain. RoPE on il2/sw2 pair tiles with cosT2/sinT2
     (rotate-half negation folded into the sw weight columns).
     qT/ropeQ spilled to internal DRAM (read back in stage 2).
  C) kv b-proj: kT blocks (normalized ckv moving) and v natural blocks
     (normalized ckv stationary). kT/ropeK/v stay resident in SBUF.

Stage 2 (per i-tile of 512, after all projections):
  D) attention, transposed-scores form: scoresT[j,i] psum with kT/ropeK
     stationary and qT/ropeQ moving; exp on ACT; causal diag blocks
     multiplied by tri masks; attnT[dv,i] accumulated with v stationary;
     row sums via all-ones stationary matmul (broadcast over partitions);
     normalization via DVE reciprocal + multiply.
  E) o-proj: o[s,:] psum accumulated over the 4 heads with attnT
     stationary and the WoT slice moving; partial output to DRAM.

SCALING and q_a layernorm weight are folded into Wqb on the host; kv
layernorm weight into Wkvb. Softmax skips max-subtraction (|scores| is
O(5) here so exp is safe in fp32; exp(s)/sum(exp(s)) == softmax exactly).
The causal mask is detected on the host; an all-ones mask uses the same
program with a full j-loop and no tri multiply.
"""

import math
import numpy as np

B, S_FULL, HID = 2, 2048, 2048
NH, Q_LORA, KV_LORA = 16, 1536, 512
D_ROPE, D_NOPE, D_V = 64, 128, 128
D_QK = D_NOPE + D_ROPE
ROPE_FACTOR, MSCALE_ALL_DIM = 40.0, 1.0
EPS = 1e-6
N_CORES, TPH = 8, 4  # TPH heads per core


def _mscale(scale, mscale):
    return 1.0 if scale <= 1 else 0.1 * mscale * math.log(scale) + 1.0


SCALING = (D_QK ** -0.5) * _mscale(ROPE_FACTOR, MSCALE_ALL_DIM) ** 2

# interleave map: il[j] = rot[2j] (j<32), il[32+j] = rot[2j+1]
IL_IDX = np.array([2 * j for j in range(32)] + [2 * j + 1 for j in range(32)])
# sw = rotate_half(il): sw[j] = -il[j+32] (j<32); sw[32+j] = il[j]
SW_SRC = np.concatenate([IL_IDX[32:], IL_IDX[:32]])
SW_SIGN = np.concatenate([-np.ones(32, np.float32), np.ones(32, np.float32)])

_NC_CACHE = {}


def _build_nc(S, ST, causal):
    """Build + compile the per-core Bass program (same NEFF for all cores)."""
    import concourse.bacc as bacc
    import concourse.tile as tile
    import concourse.mybir as mybir
    from contextlib import ExitStack

    f32 = mybir.dt.float32
    f32r = mybir.dt.float32r

    def r(ap):
        return ap.bitcast(f32r)
    Sqrt = mybir.ActivationFunctionType.Sqrt
    Exp = mybir.ActivationFunctionType.Exp

    NT = S // ST          # number of s-tiles / i-tiles
    NJC = ST // 128       # 128-chunks per s-tile
    NHC = HID // 128      # hid chunks (16)
    NRC = Q_LORA // 128   # q-lora chunks (12)
    NKC = KV_LORA // 128  # kv-lora chunks (4)
    NAB = NRC + NKC + 2   # a-proj M-blocks
    AG = 6                # a-proj psum group size
    NQB = TPH + 4         # q b-proj M-blocks: 4 nope + 2 il2 + 2 sw2

    nc = bacc.Bacc()
    hT = nc.dram_tensor("hT", [NHC, 128, S], f32r, kind="ExternalInput")
    wa = nc.dram_tensor("wa", [NHC, 128, 128 * NAB], f32r, kind="ExternalInput")
    wb = nc.dram_tensor("wb", [NRC, 128, 128 * NQB], f32r, kind="ExternalInput")
    wkv = nc.dram_tensor("wkv", [NKC, 128, 1024], f32r, kind="ExternalInput")
    wo = nc.dram_tensor("wo", [TPH, 128, HID], f32r, kind="ExternalInput")
    cs2 = nc.dram_tensor("cs2", [128, S], f32r, kind="ExternalInput")
    sn2 = nc.dram_tensor("sn2", [128, S], f32r, kind="ExternalInput")
    tri = nc.dram_tensor("tri", [128, NJC, ST], f32r, kind="ExternalInput")
    onesd = nc.dram_tensor("onesd", [128, 128], f32r, kind="ExternalInput")
    o_dram = nc.dram_tensor("o", [S, HID], f32, kind="ExternalOutput")
    # internal spill for qT (nope + roped pairs) between stage 1 and 2
    qsp = nc.dram_tensor("qsp", [NT, 128, (TPH + 2) * ST], f32r)

    hT_r = hT.rearrange("c p s -> p c s")

    with tile.TileContext(nc) as tc, ExitStack() as ctx:
        singles = ctx.enter_context(tc.tile_pool(name="singles", bufs=1))

        # ---- constants / residents ----
        ones = singles.tile([128, 128], f32r)
        nc.gpsimd.dma_start(out=ones, in_=onesd[:, :])
        epsb = singles.tile([128, 1], f32)
        nc.vector.memset(epsb, EPS)
        kTn = singles.tile([128, TPH, S], f32r)      # k_nope.T normalized
        ropeK = singles.tile([128, S], f32r)         # roped k_rot (dup halves)
        v_nat = singles.tile([128, S // 128, TPH * D_V], f32r)

        # ================= STAGE 1: projections =================
        s1ctx = ExitStack()
        p_hts = s1ctx.enter_context(tc.tile_pool(name="hts", bufs=1))
        for t in range(NT):
            s0 = t * ST
            ssl = slice(s0, s0 + ST)
            with ExitStack() as tctx:
                p_tile = tctx.enter_context(
                    tc.tile_pool(name=f"ptile{t}", bufs=1))
                rawq = p_tile.tile([128, NRC, ST], f32r, tag="rawq")
                nckv = p_tile.tile([128, NKC, ST], f32r, tag="nckv")
                kil = p_tile.tile([128, ST], f32r, tag="kil")
                ksw = p_tile.tile([128, ST], f32r, tag="ksw")
                rs_q = p_tile.tile([128, ST], f32, tag="rsq")
                rs_kv = p_tile.tile([128, ST], f32, tag="rskv")
                cs_t = p_tile.tile([128, ST], f32r, tag="cs")
                sn_t = p_tile.tile([128, ST], f32r, tag="sn")
                nc.gpsimd.dma_start(out=cs_t, in_=cs2[:, ssl])
                nc.gpsimd.dma_start(out=sn_t, in_=sn2[:, ssl])

                hts = p_hts.tile([128, NHC, ST], f32r, tag="hts")
                for hq in range(4):
                    nc.scalar.dma_start(
                        out=hts[:, hq * (NHC // 4):(hq + 1) * (NHC // 4), :],
                        in_=hT_r[:, hq * (NHC // 4):(hq + 1) * (NHC // 4), ssl])

                # ---------- phase A ----------
                with ExitStack() as actx:
                    p_wa = actx.enter_context(
                        tc.tile_pool(name=f"wa{t}", bufs=3))
                    p_sq = actx.enter_context(
                        tc.tile_pool(name=f"sq{t}", bufs=3))
                    ps_mm = actx.enter_context(
                        tc.tile_pool(name=f"psA{t}", bufs=AG, space="PSUM"))
                    ps_ss = actx.enter_context(
                        tc.tile_pool(name=f"psS{t}", bufs=1, space="PSUM"))

                    ss_q = ps_ss.tile([128, ST], f32, tag="ssq")
                    ss_kv = ps_ss.tile([128, ST], f32, tag="sskv")

                    n_groups = (NAB + AG - 1) // AG
                    for mg in range(n_groups):
                        blocks = list(range(mg * AG, min((mg + 1) * AG, NAB)))
                        psums = [ps_mm.tile([128, ST], f32, tag="aproj",
                                            name=f"apr{t}_{mg}_{bi}")
                                 for bi in range(len(blocks))]
                        for hc in range(NHC):
                            wt = p_wa.tile([128, len(blocks) * 128], f32r,
                                           tag="wa")
                            nc.sync.dma_start(
                                out=wt,
                                in_=wa[hc, :,
                                       blocks[0] * 128:(blocks[-1] + 1) * 128])
                            for bi in range(len(blocks)):
                                nc.tensor.matmul(
                                    psums[bi],
                                    r(wt[:, bi * 128:(bi + 1) * 128]),
                                    r(hts[:, hc, :]),
                                    start=(hc == 0), stop=(hc == NHC - 1))
                        for bi, m in enumerate(blocks):
                            if m < NRC:           # q_a block
                                nc.vector.tensor_copy(rawq[:, m, :], psums[bi])
                                sqt = p_sq.tile([128, ST], f32r, tag="sq")
                                nc.scalar.square(sqt, psums[bi])
                                nc.tensor.matmul(
                                    ss_q, r(ones[:, :]), r(sqt[:, :]),
                                    start=(m == 0), stop=(m == NRC - 1))
                            elif m < NRC + NKC:   # ckv block
                                mk = m - NRC
                                nc.vector.tensor_copy(nckv[:, mk, :], psums[bi])
                                sqt = p_sq.tile([128, ST], f32r, tag="sq")
                                nc.scalar.square(sqt, psums[bi])
                                nc.tensor.matmul(
                                    ss_kv, r(ones[:, :]), r(sqt[:, :]),
                                    start=(mk == 0), stop=(mk == NKC - 1))
                            elif m == NRC + NKC:  # k-rope il (dup)
                                nc.vector.tensor_copy(kil, psums[bi])
                            else:                 # k-rope sw (dup, negated)
                                nc.vector.tensor_copy(ksw, psums[bi])

                    # rmsnorm scales (broadcast tiles, partitions identical)
                    nc.scalar.activation(rs_q, ss_q, Sqrt,
                                         bias=epsb, scale=1.0 / Q_LORA)
                    nc.vector.reciprocal(rs_q, rs_q)
                    nc.scalar.activation(rs_kv, ss_kv, Sqrt,
                                         bias=epsb, scale=1.0 / KV_LORA)
                    nc.vector.reciprocal(rs_kv, rs_kv)

                    for mk in range(NKC):
                        nc.vector.tensor_mul(nckv[:, mk, :], nckv[:, mk, :],
                                             rs_kv)

                    # K rope into resident ropeK (in-place muls)
                    nc.vector.tensor_mul(kil, kil, cs_t)
                    nc.vector.tensor_mul(ksw, ksw, sn_t)
                    nc.vector.tensor_add(ropeK[:, ssl], kil, ksw)

                # ---------- phase B (weights streamed per r-chunk) ----------
                with ExitStack() as bctx:
                    p_wb = bctx.enter_context(
                        tc.tile_pool(name=f"wb{t}", bufs=3))
                    p_qt = bctx.enter_context(
                        tc.tile_pool(name=f"qt{t}", bufs=1))
                    p_qtmp = bctx.enter_context(
                        tc.tile_pool(name=f"qtmp{t}", bufs=1))
                    ps_qb = bctx.enter_context(
                        tc.tile_pool(name=f"psB{t}", bufs=NQB, space="PSUM"))

                    qTn = p_qt.tile([128, TPH, ST], f32r, tag="qtn")
                    ropeQ = p_qt.tile([128, 2, ST], f32r, tag="ropeq")
                    il2 = p_qtmp.tile([128, 2, ST], f32r, tag="il2")
                    sw2 = p_qtmp.tile([128, 2, ST], f32r, tag="sw2")
                    pqs = [ps_qb.tile([128, ST], f32, tag="qb",
                                      name=f"pq{t}_{m}") for m in range(NQB)]
                    for rc in range(NRC):
                        wb_rc = p_wb.tile([128, 128 * NQB], f32r, tag="wb")
                        nc.scalar.dma_start(out=wb_rc, in_=wb[rc, :, :])
                        for m in range(NQB):
                            nc.tensor.matmul(
                                pqs[m], r(wb_rc[:, m * 128:(m + 1) * 128]),
                                r(rawq[:, rc, :]),
                                start=(rc == 0), stop=(rc == NRC - 1))
                    for m in range(NQB):
                        pq = pqs[m]
                        if m < 4:
                            nc.vector.tensor_mul(qTn[:, m, :], pq, rs_q)
                        elif m < 6:
                            nc.vector.tensor_mul(il2[:, m - 4, :], pq, rs_q)
                        else:
                            nc.vector.tensor_mul(sw2[:, m - 6, :], pq, rs_q)

                    for p in range(2):
                        nc.vector.tensor_mul(il2[:, p, :], il2[:, p, :], cs_t)
                        nc.vector.tensor_mul(sw2[:, p, :], sw2[:, p, :], sn_t)
                        nc.vector.tensor_add(ropeQ[:, p, :], il2[:, p, :],
                                             sw2[:, p, :])

                    # spill qT for stage 2
                    qsp_t = qsp[t].rearrange("p (c s) -> p c s", s=ST)
                    nc.gpsimd.dma_start(out=qsp_t[:, 0:TPH, :], in_=qTn)
                    nc.gpsimd.dma_start(out=qsp_t[:, TPH:TPH + 2, :], in_=ropeQ)

                # ---------- phase C (weights streamed per r-chunk) ----------
                with ExitStack() as cctx:
                    p_wkv = cctx.enter_context(
                        tc.tile_pool(name=f"wkv{t}", bufs=2))
                    ps_kv = cctx.enter_context(
                        tc.tile_pool(name=f"psC{t}", bufs=TPH + NJC,
                                     space="PSUM"))

                    pks = [ps_kv.tile([128, ST], f32, tag="kvb",
                                      name=f"pk{t}_{hl}") for hl in range(TPH)]
                    pvs = [ps_kv.tile([128, TPH * D_V], f32, tag="kvb",
                                      name=f"pv{t}_{sb}") for sb in range(NJC)]
                    for rc in range(NKC):
                        wkv_rc = p_wkv.tile([128, 1024], f32r, tag="wkv")
                        nc.scalar.dma_start(out=wkv_rc, in_=wkv[rc, :, :])
                        for hl in range(TPH):
                            nc.tensor.matmul(
                                pks[hl], r(wkv_rc[:, hl * 128:(hl + 1) * 128]),
                                r(nckv[:, rc, :]),
                                start=(rc == 0), stop=(rc == NKC - 1))
                        for sb in range(NJC):
                            nc.tensor.matmul(
                                pvs[sb], r(nckv[:, rc, sb * 128:(sb + 1) * 128]),
                                r(wkv_rc[:, 512:1024]),
                                start=(rc == 0), stop=(rc == NKC - 1))
                    for hl in range(TPH):
                        nc.vector.tensor_copy(kTn[:, hl, ssl], pks[hl])
                    for sb in range(NJC):
                        nc.vector.tensor_copy(v_nat[:, t * NJC + sb, :], pvs[sb])

        s1ctx.close()

        # ================= STAGE 2: attention + o-proj =================
        with ExitStack() as sctx:
            p_ql = sctx.enter_context(tc.tile_pool(name="ql", bufs=2))
            p_exp = sctx.enter_context(tc.tile_pool(name="exp", bufs=3))
            p_at = sctx.enter_context(tc.tile_pool(name="at", bufs=1))
            p_rec = sctx.enter_context(tc.tile_pool(name="rec", bufs=2))
            p_wo = sctx.enter_context(tc.tile_pool(name="wo", bufs=4))
            p_oo = sctx.enter_context(tc.tile_pool(name="oo", bufs=3))
            ps_sc = sctx.enter_context(
                tc.tile_pool(name="ps_sc", bufs=2, space="PSUM"))
            ps_av = sctx.enter_context(
                tc.tile_pool(name="ps_av", bufs=2, space="PSUM"))
            ps_o = sctx.enter_context(
                tc.tile_pool(name="ps_o", bufs=2, space="PSUM"))

            tri_sb = p_ql.tile([128, NJC, ST], f32r, tag="tri", bufs=1)
            nc.gpsimd.dma_start(out=tri_sb, in_=tri[:, :, :])

            wo_tiles = []
            for hl in range(TPH):
                wo_sb = p_wo.tile([128, HID], f32r, tag="wo",
                                  name=f"wo{hl}")
                nc.gpsimd.dma_start(out=wo_sb, in_=wo[hl, :, :])
                wo_tiles.append(wo_sb)

            for t in range(NT):
                s0 = t * ST
                qsp_t = qsp[t].rearrange("p (c s) -> p c s", s=ST)
                qt_sb = p_ql.tile([128, TPH + 2, ST], f32r, tag="ql")
                nc.scalar.dma_start(out=qt_sb, in_=qsp_t)

                n_jc = (t + 1) * NJC if causal else S // 128
                attnT = [None] * TPH
                for hl in range(TPH):
                    l, pr = hl % 2, hl // 2
                    p_attn = ps_av.tile([128, ST], f32, tag="attn")
                    p_rows = ps_av.tile([128, ST], f32, tag="rows")
                    for jc in range(n_jc):
                        psc = ps_sc.tile([128, ST], f32, tag="score")
                        nc.tensor.matmul(
                            psc, r(kTn[:, hl, jc * 128:(jc + 1) * 128]),
                            r(qt_sb[:, hl, :]), start=True, stop=False)
                        nc.tensor.matmul(
                            psc,
                            r(ropeK[64 * l:64 * (l + 1),
                                    jc * 128:(jc + 1) * 128]),
                            r(qt_sb[64 * l:64 * (l + 1), TPH + pr, :]),
                            start=False, stop=True)
                        ex = p_exp.tile([128, ST], f32r, tag="exp")
                        nc.scalar.activation(ex, psc, Exp)
                        if causal and jc >= t * NJC:
                            nc.vector.tensor_mul(ex, ex,
                                                 tri_sb[:, jc - t * NJC, :])
                        nc.tensor.matmul(
                            p_attn, r(v_nat[:, jc, hl * D_V:(hl + 1) * D_V]),
                            r(ex[:, :]),
                            start=(jc == 0), stop=(jc == n_jc - 1))
                        nc.tensor.matmul(
                            p_rows, r(ones[:, :]), r(ex[:, :]),
                            start=(jc == 0), stop=(jc == n_jc - 1))
                    rec = p_rec.tile([128, ST], f32, tag="rec")
                    nc.vector.reciprocal(rec, p_rows)
                    at = p_at.tile([128, ST], f32r, tag=f"at{hl}")
                    nc.vector.tensor_mul(at, p_attn, rec)
                    attnT[hl] = at

                for sb in range(NJC):
                    for nt2 in range(HID // 512):
                        po = ps_o.tile([128, 512], f32, tag="o")
                        for hl in range(TPH):
                            nc.tensor.matmul(
                                po, r(attnT[hl][:, sb * 128:(sb + 1) * 128]),
                                r(wo_tiles[hl][:, nt2 * 512:(nt2 + 1) * 512]),
                                start=(hl == 0), stop=(hl == TPH - 1))
                        oo = p_oo.tile([128, 512], f32, tag="oo")
                        nc.vector.tensor_copy(oo, po)
                        nc.gpsimd.dma_start(
                            out=o_dram[s0 + sb * 128:s0 + (sb + 1) * 128,
                                       nt2 * 512:(nt2 + 1) * 512],
                            in_=oo)

    nc.compile()
    return nc


def _prep_core_inputs(S, ST, core, hidden_states, cos, sin,
                      Wqa, q_a_ln_w, Wqb, Wkva, kv_a_ln_w, Wkvb, Wo):
    """Host-side shard + layout prep for one core."""
    NJC = ST // 128
    NHC = HID // 128
    NRC = Q_LORA // 128
    NKC = KV_LORA // 128
    NAB = NRC + NKC + 2

    b = core // TPH
    hset = core % TPH
    hgs = [TPH * hset + hl for hl in range(TPH)]

    f32 = np.float32
    out = {}

    out["hT"] = np.ascontiguousarray(
        hidden_states[b].T.reshape(NHC, 128, S)).astype(f32)

    # ---- wa: [NHC, 128, 128*NAB] ----
    wa_cols = np.empty((HID, 128 * NAB), f32)
    wa_cols[:, :Q_LORA] = Wqa.T
    wa_cols[:, Q_LORA:Q_LORA + KV_LORA] = Wkva[:KV_LORA].T
    il_block = Wkva[KV_LORA + IL_IDX].T          # [HID, 64]
    sw_block = (SW_SIGN[:, None] * Wkva[KV_LORA + SW_SRC]).T
    c0 = Q_LORA + KV_LORA
    wa_cols[:, c0:c0 + 64] = il_block
    wa_cols[:, c0 + 64:c0 + 128] = il_block
    wa_cols[:, c0 + 128:c0 + 192] = sw_block
    wa_cols[:, c0 + 192:c0 + 256] = sw_block
    out["wa"] = np.ascontiguousarray(wa_cols.reshape(NHC, 128, 128 * NAB))

    # ---- wb: [NRC, 128, 1024] (nope 512 | il2 256 | sw2 256) ----
    Wqbp = (Wqb * SCALING) * q_a_ln_w[None, :]   # [NH*192, Q_LORA]
    wb_cols = np.empty((Q_LORA, 1024), f32)
    for hl in range(TPH):
        hg = hgs[hl]
        wb_cols[:, hl * 128:(hl + 1) * 128] = Wqbp[192 * hg:192 * hg + 128].T
        p, l = hl // 2, hl % 2
        rot = Wqbp[192 * hg + 128:192 * hg + 192]     # [64, Q_LORA]
        wb_cols[:, 512 + 128 * p + 64 * l:512 + 128 * p + 64 * (l + 1)] = \
            rot[IL_IDX].T
        wb_cols[:, 768 + 128 * p + 64 * l:768 + 128 * p + 64 * (l + 1)] = \
            (SW_SIGN[:, None] * rot[SW_SRC]).T
    out["wb"] = np.ascontiguousarray(wb_cols.reshape(NRC, 128, 1024))

    # ---- wkv: [NKC, 128, 1024] (kT 512 | v 512) ----
    Wkvbp = Wkvb * kv_a_ln_w[None, :]            # [NH*256, KV_LORA]
    wkv_cols = np.empty((KV_LORA, 1024), f32)
    for hl in range(TPH):
        hg = hgs[hl]
        wkv_cols[:, hl * 128:(hl + 1) * 128] = Wkvbp[256 * hg:256 * hg + 128].T
        wkv_cols[:, 512 + hl * 128:512 + (hl + 1) * 128] = \
            Wkvbp[256 * hg + 128:256 * hg + 256].T
    out["wkv"] = np.ascontiguousarray(wkv_cols.reshape(NKC, 128, 1024))

    # ---- wo: [TPH, 128, HID] ----
    wo_arr = np.empty((TPH, 128, HID), f32)
    for hl in range(TPH):
        hg = hgs[hl]
        wo_arr[hl] = Wo[:, 128 * hg:128 * (hg + 1)].T
    out["wo"] = np.ascontiguousarray(wo_arr)

    out["onesd"] = np.ones((128, 128), f32)

    # ---- cos/sin dup tiles [128, S] ----
    j = np.arange(128) % 64
    out["cs2"] = np.ascontiguousarray(cos[b].T[j]).astype(f32)
    out["sn2"] = np.ascontiguousarray(sin[b].T[j]).astype(f32)

    # ---- tri [128, NJC, ST]: keep iff i_local >= 128*o + j_local ----
    jr = np.arange(128)[:, None, None]
    o_ = np.arange(NJC)[None, :, None]
    ic = np.arange(ST)[None, None, :]
    out["tri"] = (ic >= 128 * o_ + jr).astype(f32)

    return out


class _Exec:
    """Cached PJRT executable for the SPMD kernel (axon path).

    Mirrors concourse.bass2jax.run_bass_via_pjrt but without output-buffer
    donation (this kernel writes every output element) so the jitted fn can
    be re-invoked with device-resident args for timing.
    """

    def __init__(self, nc):
        import jax
        import concourse.mybir as mybir
        from jax.sharding import Mesh, PartitionSpec
        from concourse.bass2jax import (_bass_exec_p, install_neuronx_cc_hook,
                                        partition_id_tensor)
        try:
            from jax.experimental.shard_map import shard_map
        except ImportError:
            from jax import shard_map

        install_neuronx_cc_hook()
        pname = nc.partition_id_tensor.name if nc.partition_id_tensor else None
        in_names, out_names, out_avals, zero_outs = [], [], [], []
        for alloc in nc.m.functions[0].allocations:
            if not isinstance(alloc, mybir.MemoryLocationSet):
                continue
            name = alloc.memorylocations[0].name
            if alloc.kind == "ExternalInput":
                if name != pname:
                    in_names.append(name)
            elif alloc.kind == "ExternalOutput":
                out_names.append(name)
                shape = tuple(alloc.tensor_shape)
                dtype = mybir.dt.np(alloc.dtype)
                out_avals.append(jax.core.ShapedArray(shape, dtype))
                zero_outs.append(np.zeros(shape, dtype))
        self.in_names = list(in_names)
        self.out_names = out_names
        self.out_avals = out_avals
        self.zero_outs = zero_outs
        n_params = len(in_names)
        all_names = in_names + out_names
        if pname is not None:
            all_names = all_names + [pname]

        def _body(*args):
            operands = list(args)
            if pname is not None:
                operands.append(partition_id_tensor())
            outs = _bass_exec_p.bind(
                *operands,
                out_avals=tuple(out_avals),
                in_names=tuple(all_names),
                out_names=tuple(out_names),
                lowering_input_output_aliases=(),
                sim_require_finite=True,
                sim_require_nnan=True,
                nc=nc,
            )
            return tuple(outs)

        devices = jax.devices()[:N_CORES]
        self.mesh = Mesh(np.asarray(devices), ("core",))
        in_specs = (PartitionSpec("core"),) * (n_params + len(out_names))
        out_specs = (PartitionSpec("core"),) * len(out_names)
        self.fn = jax.jit(
            shard_map(_body, mesh=self.mesh, in_specs=in_specs,
                      out_specs=out_specs, check_rep=False),
            keep_unused=True)
        self.n_params = n_params

    def concat_args(self, in_maps):
        per_core = [[np.asarray(m[n]) for n in self.in_names] for m in in_maps]
        concat_in = [
            np.concatenate([per_core[c][i] for c in range(N_CORES)], axis=0)
            for i in range(self.n_params)]
        concat_zeros = [
            np.zeros((N_CORES * z.shape[0], *z.shape[1:]), z.dtype)
            for z in self.zero_outs]
        return concat_in + concat_zeros

    def run(self, in_maps):
        args = self.concat_args(in_maps)
        out_arrs = self.fn(*args)
        return [
            {n: np.asarray(out_arrs[i]).reshape(N_CORES,
                                                *self.out_avals[i].shape)[c]
             for i, n in enumerate(self.out_names)}
            for c in range(N_CORES)]


_EXEC_CACHE = {}


def _get_exec(key):
    if key not in _EXEC_CACHE:
        if key not in _NC_CACHE:
            _NC_CACHE[key] = _build_nc(*key)
        _EXEC_CACHE[key] = _Exec(_NC_CACHE[key])
    return _EXEC_CACHE[key]


def kernel(hidden_states, cos, sin, attention_mask, Wqa, q_a_ln_w, Wqb,
           Wkva, kv_a_ln_w, Wkvb, Wo):

    S = hidden_states.shape[1]
    ST = 512
    mask = np.asarray(attention_mask).reshape(S, S)
    causal = bool(np.array_equal(mask, np.tril(np.ones((S, S), bool))))
    if not causal:
        assert mask.all(), "only causal or all-ones masks supported"

    ex = _get_exec((S, ST, causal))

    args = [np.asarray(hidden_states, np.float32), np.asarray(cos, np.float32),
            np.asarray(sin, np.float32), np.asarray(Wqa, np.float32),
            np.asarray(q_a_ln_w, np.float32), np.asarray(Wqb, np.float32),
            np.asarray(Wkva, np.float32), np.asarray(kv_a_ln_w, np.float32),
            np.asarray(Wkvb, np.float32), np.asarray(Wo, np.float32)]
    in_maps = [_prep_core_inputs(S, ST, core, *args) for core in range(N_CORES)]

    results = ex.run(in_maps)

    out = np.zeros((B, S, HID), np.float32)
    for core in range(N_CORES):
        out[core // TPH] += results[core]["o"]
    return out

